# revision 13
# baseline (speedup 1.0000x reference)
"""Multi-head attention kernel for 8 Trainium2 NeuronCores — v2.

Problem: nn_MultiHeadAttention_49246095016569
  q,k,v: [S=2048, B=2, E=512] f32; per-head projections Wq/Wk/Wv [64,64],
  output FC Wfc [512,512] + bfc [512].  The reference's plain reshape makes
  each (b,h) pair a contiguous [2048,64] chunk; 16 chunks, 2 per core.

v2 architecture (per chunk, qc/kc/vc = [2048,64]):
  * Host folds:  G = Wq.T @ Wk / 8   (both QK projections + 1/sqrt(d))
                 M_j = Wv.T @ Wfc.T[64j:64j+64]  (Wv folded into FC weights)
    so neither K nor V needs any on-chip projection.
  * q,k loaded via hardware DMA-transpose of the [1024,128]-fused view:
    kf/qf [128,1024]: rows 0:64 = d-lanes of EVEN rows, rows 64:128 = ODD
    rows (parity split).  v loaded raw in "slot" layout
    vp[p, u, 0:64] = v[1024*(u//8) + 8p + (u%8)], vp[p, u, 64] = 1.
  * qp = q @ G computed once; [128,2,1024] = (parity, fused col), written
    to both partition halves so both PE row-groups can stream it.
  * Scores: row-tiled concurrent matmul pairs — tile_position (0,0) and
    (64,0) run an even-k-subset and an odd-k-subset [64x128] weight tile
    through the PE array simultaneously (K=64 row tiling, ~1.9x).
    k-subsets are strided column views of kf; subset u matches vp slot u.
  * exp(S/8 - 2): ACT for most tiles (exact, fp16 out); a few tiles go to
    DVE+GPSIMD via the Schraudolph bit-hack (y = S8*1477.3 + 12449.6,
    clamp, cast u16, bitcast fp16 ~= 2^x).  The -2 offset cancels in
    softmax; ~3% hack ripple on ~19% of mass adds ~1% output error.
  * PV: fp16 matmuls [128,65]x[128,512] accumulating [65,512]; the ones
    column gives softmax denominators for free.
  * Normalize: reciprocal_approx_fast on the PSUM accumulator, PE ones-
    broadcast of the recip row, one multiply into atT (fp16).
  * FC: out = atT.T @ M with K=65: the ones row adds the bias for free.
"""

import numpy as np

import concourse.bass as bass
import concourse.mybir as mybir
import concourse.tile as tile
from concourse import bacc
from concourse import bass_utils

F32 = mybir.dt.float32
F32R = mybir.dt.float32r
F16 = mybir.dt.float16
U16 = mybir.dt.uint16
ACT_EXP = mybir.ActivationFunctionType.Exp
AOP = mybir.AluOpType

S = 2048
D = 64
E = 512
NCORES = 8
NCHUNK = 2
J = S // 2  # fused column count per chunk

# Schraudolph constants: fp16 bits of exp(S8-2) ~= S8*C1 + C2
BH_C1 = 1024.0 * 1.4426950408889634                   # 1477.32
BH_C2 = 1024.0 * (15.0 - 0.0430) - 2.0 * BH_C1        # 12361.4
BH_MAX = 31743.0  # max finite fp16 bit pattern

# pairs (of 16 per fused-q-block) whose exp runs on DVE+GPSIMD instead of ACT
DVE_PAIRS = frozenset({0, 1, 2, 3, 4})
N_WARM = 10  # dummy matmuls at start to lift the HAM clock gate


def build_core_program():
    nc = bacc.Bacc(trn_type="TRN2")

    q_in = nc.dram_tensor("q_in", (NCHUNK * S, D), F16, kind="ExternalInput")
    k_in = nc.dram_tensor("k_in", (NCHUNK * S, D), F16, kind="ExternalInput")
    v_in = nc.dram_tensor("v_in", (NCHUNK * S, D), F16, kind="ExternalInput")
    g_in = nc.dram_tensor("g_in", (D, D), F16, kind="ExternalInput")
    m_in = nc.dram_tensor("m_in", (D + 1, 8 * E), F16, kind="ExternalInput")
    out = nc.dram_tensor("out", (NCHUNK * 256, E), F32, kind="ExternalOutput")

    with tile.TileContext(nc) as tc:
        with (
            tc.tile_pool(name="consts", bufs=1) as consts,
            tc.tile_pool(name="kqf", bufs=2) as kqf_pool,
            tc.tile_pool(name="qp", bufs=2) as qp_pool,
            tc.tile_pool(name="vp", bufs=2) as vp_pool,
            tc.tile_pool(name="at", bufs=2) as at_pool,
            tc.tile_pool(name="pp", bufs=8) as p_pool,
            tc.tile_pool(name="ys", bufs=2) as y_pool,
            tc.tile_pool(name="rs", bufs=2) as rs_pool,
            tc.tile_pool(name="rb", bufs=2) as rb_pool,
            tc.tile_pool(name="ob", bufs=2) as out_pool,
            tc.tile_pool(name="ps_score", bufs=2, space="PSUM") as ps_score,
            tc.tile_pool(name="ps_misc", bufs=4, space="PSUM") as ps_misc,
        ):
            # ---- constants (issued on the ACT HWDGE ring; q/k transposes
            # go on the Sync ring so the two sets of transfers overlap) ----
            g2 = consts.tile([128, D], F16)
            nc.scalar.dma_start(g2[0:64, :], g_in[:])
            nc.scalar.dma_start(g2[64:128, :], g_in[:])
            m_sb = consts.tile([D + 1, 8, E], F16)
            nc.scalar.dma_start(m_sb[:], m_in[:].rearrange("p (j e) -> p j e", j=8))
            ones65 = consts.tile([D + 1, D], F16)
            nc.vector.memset(ones65[:], 1.0)
            bias_t = consts.tile([128, 1], F32)
            nc.vector.memset(bias_t[:], -2.0)

            # ---- HAM warm-up: real (zero-data) matmuls while first DMAs land
            zw = consts.tile([128, 128 + E], F16)
            nc.vector.memset(zw[:], 0.0)
            warm_ps = ps_score.tile([128, 1024], F32, tag="score")
            for _ in range(N_WARM):
                nc.tensor.matmul(
                    warm_ps[:, 0:E], zw[:, 0:128], zw[:, 128:], start=True, stop=True
                )

            def emit_chunk_loads(c):
                co = c * S
                kf = kqf_pool.tile([128, J], F16, tag="kf")
                qf = kqf_pool.tile([128, J], F16, tag="qf")
                nc.sync.dma_start(
                    qf[:],
                    q_in[co : co + S, :].rearrange("(a b) d -> a (b d)", b=2),
                    transpose=True,
                )
                nc.sync.dma_start(
                    kf[:],
                    k_in[co : co + S, :].rearrange("(a b) d -> a (b d)", b=2),
                    transpose=True,
                )
                vp = vp_pool.tile([128, 16, D + 1], F16, tag="vp")
                for hl in range(2):
                    nc.scalar.dma_start(
                        vp[:, 8 * hl : 8 * (hl + 1), 0:D],
                        v_in[co + 1024 * hl : co + 1024 * (hl + 1), :].rearrange(
                            "(p t) d -> p t d", p=128
                        ),
                    )
                nc.gpsimd.memset(vp[:, :, D : D + 1], 1.0)
                return kf, qf, vp

            def emit_qp(c, qf):
                # qp2[r, par, j] = (q @ G)[2j+par, :] for both partition halves
                qp2 = qp_pool.tile([128, 2, J], F16, tag="qp2")
                for jh in range(2):
                    pq = ps_score.tile([128, 1024], F32, tag="score")
                    nc.tensor.matmul(
                        pq[0:64, 0:512],
                        g2[0:64, :],
                        qf[0:64, 512 * jh : 512 * (jh + 1)],
                        start=True,
                        stop=True,
                        tile_position=(0, 0),
                    )
                    nc.tensor.matmul(
                        pq[0:64, 512:1024],
                        g2[64:128, :],
                        qf[64:128, 512 * jh : 512 * (jh + 1)],
                        start=True,
                        stop=True,
                        tile_position=(64, 0),
                    )
                    pqv = pq[0:64, :].rearrange("d (par j) -> d par j", par=2)
                    nc.vector.tensor_copy(
                        qp2[0:64, :, 512 * jh : 512 * (jh + 1)], pqv
                    )
                    nc.scalar.copy(
                        qp2[64:128, :, 512 * jh : 512 * (jh + 1)], pqv
                    )
                return qp2

            def subset_ap(kf, half, u):
                # [64,128] weight tile: column p' of the tile is kf column
                # j = 512*(u//8) + 4*p' + (u%8)//2 in partition rows `half`
                base = kf[64 * half : 64 * half + 64, :]
                v = base.rearrange("d (hl p t) -> d hl t p", hl=2, t=4)
                return v[:, u // 8, (u % 8) // 2, :]

            def emit_pairs(st8, prange):
                c, fqb, kf, qp2, vp, atT, pavs, deferred = st8
                jb = 512 * fqb
                for p in prange:
                    g = p % 8
                    swap = p >= 8
                    parA = 1 if swap else 0
                    # subset indices: psum half 0 always holds par0's block
                    u0 = 2 * g + 1 if swap else 2 * g      # -> par 0
                    u1 = 2 * g if swap else 2 * g + 1      # -> par 1
                    st = ps_score.tile([128, 1024], F32, tag="score")
                    hA = 512 if swap else 0
                    nc.tensor.matmul(
                        st[:, hA : hA + 512],
                        subset_ap(kf, 0, 2 * g),
                        qp2[0:64, parA, jb : jb + 512],
                        start=True,
                        stop=True,
                        tile_position=(0, 0),
                    )
                    nc.tensor.matmul(
                        st[:, 512 - hA : 1024 - hA],
                        subset_ap(kf, 1, 2 * g + 1),
                        qp2[64:128, 1 - parA, jb : jb + 512],
                        start=True,
                        stop=True,
                        tile_position=(64, 0),
                    )
                    pt = p_pool.tile([128, 2, 512], F16, tag="pt")
                    if p in DVE_PAIRS:
                        ysb = y_pool.tile([128, 1024], F32, tag="ys")
                        nc.vector.tensor_scalar(
                            ysb[:], st[:], BH_C1, BH_C2, AOP.mult, AOP.add
                        )
                        yv = ysb[:].rearrange("k (a f) -> k a f", a=2)
                        nc.gpsimd.tensor_scalar(
                            pt[:].bitcast(U16), yv, BH_MAX, 0.0, AOP.min, AOP.max
                        )
                        deferred.append((pt, u0, u1))
                    else:
                        nc.scalar.activation(pt[:], st[:], ACT_EXP, bias=bias_t[:])
                        first = p == min(q for q in range(16) if q not in DVE_PAIRS)
                        if first:
                            pav0 = ps_misc.tile([D + 1, 512], F32, tag="misc2")
                            pav1 = ps_misc.tile([D + 1, 512], F32, tag="misc2")
                            pavs[0], pavs[1] = pav0, pav1
                            st8[6] = pavs
                        for par, u in ((0, u0), (1, u1)):
                            nc.tensor.matmul(
                                pavs[par][:],
                                vp[:, u, :],
                                pt[:, par, :],
                                start=first,
                                stop=False,
                            )

            def emit_deferred(st8):
                c, fqb, kf, qp2, vp, atT, pavs, deferred = st8
                for i, (pt, u0, u1) in enumerate(deferred):
                    for par, u in ((0, u0), (1, u1)):
                        nc.tensor.matmul(
                            pavs[par][:],
                            vp[:, u, :],
                            pt[:, par, :],
                            start=False,
                            stop=(i == len(deferred) - 1),
                        )

            def emit_norm(st8):
                # normalize both parities into atT: recip on DVE, PE ones-
                # broadcast of the recip row, fused multiply back on DVE
                c, fqb, kf, qp2, vp, atT, pavs, deferred = st8
                jb = 512 * fqb
                atv = atT[0:64, :].rearrange("d (j two) -> d two j", two=2)
                for par in (0, 1):
                    rs = rs_pool.tile([D + 1, 512], F32, tag="rs")
                    nc.vector.reciprocal_approx_fast(rs[:], pavs[par][:])
                    rsh = rs_pool.tile([D + 1, 512], F16, tag="rsh")
                    nc.vector.tensor_copy(rsh[64:65, :], rs[64:65, :])
                    rbp = ps_misc.tile([D, 512], F32, tag="misc2")
                    nc.tensor.matmul(
                        rbp[:],
                        ones65[64:65, :],
                        rsh[64:65, :],
                        start=True,
                        stop=True,
                        tile_position=(64, 0),
                    )
                    rb = rb_pool.tile([D, 512], F32, tag="rb")
                    nc.vector.tensor_copy(rb[:], rbp[:])
                    nc.vector.tensor_mul(
                        atv[:, par, jb : jb + 512], pavs[par][0:64, :], rb[:]
                    )

            def emit_fc(c, m, atT):
                atv = atT[:].rearrange("d (m r j) -> d m j r", m=2, j=8)
                po = ps_misc.tile([128, E], F32, tag="misc2")
                for j in range(8):
                    nc.tensor.matmul(
                        po[:],
                        atv[:, m, j, :],
                        m_sb[:, j, :],
                        start=(j == 0),
                        stop=(j == 7),
                    )
                ot = out_pool.tile([128, E], F32, tag="out")
                nc.vector.tensor_copy(ot[:], po[:])
                nc.sync.dma_start(
                    out[256 * c + 128 * m : 256 * c + 128 * (m + 1), :], ot[:]
                )

            loads = [emit_chunk_loads(c) for c in range(NCHUNK)]
            chunk_state = {}

            def make_state(c, fqb):
                if c not in chunk_state:
                    kf, qf, vp = loads[c]
                    qp2 = emit_qp(c, qf)
                    atT = at_pool.tile([D + 1, S], F16, tag="atT")
                    nc.gpsimd.memset(atT[64:65, :], 1.0)
                    chunk_state[c] = (kf, qp2, vp, atT)
                kf, qp2, vp, atT = chunk_state[c]
                return [c, fqb, kf, qp2, vp, atT, {}, []]

            fqbs = [(c, f) for c in range(NCHUNK) for f in range(2)]
            prev = None
            for c, f in fqbs:
                st8 = make_state(c, f)
                emit_pairs(st8, range(0, 3))
                if prev is not None:
                    emit_norm(prev)
                emit_pairs(st8, range(3, 8))
                if prev is not None:
                    emit_fc(prev[0], prev[1], prev[5])
                emit_pairs(st8, range(8, 16))
                emit_deferred(st8)
                prev = st8
            emit_norm(prev)
            emit_fc(prev[0], prev[1], prev[5])

    nc.compile()
    return nc


_NC_CACHE = None


def _get_nc():
    global _NC_CACHE
    if _NC_CACHE is None:
        _NC_CACHE = build_core_program()
    return _NC_CACHE


def make_in_maps(q, k, v, Wq, Wk, Wv, Wfc, bfc):
    q = np.ascontiguousarray(q, dtype=np.float32)
    k = np.ascontiguousarray(k, dtype=np.float32)
    v = np.ascontiguousarray(v, dtype=np.float32)
    Wq = np.asarray(Wq, np.float32)
    Wk = np.asarray(Wk, np.float32)
    Wv = np.asarray(Wv, np.float32)
    Wfc = np.asarray(Wfc, np.float32)

    g = ((Wq.T @ Wk) / 8.0).astype(np.float16)  # [64,64]: qp = q @ g
    m = np.zeros((D + 1, 8, E), dtype=np.float16)
    wfct = Wfc.T  # [512, 512]
    for j in range(8):
        m[0:64, j, :] = (Wv.T @ wfct[64 * j : 64 * (j + 1), :]).astype(np.float16)
    m[64, 0, :] = np.asarray(bfc, np.float32).astype(np.float16)
    m2 = np.ascontiguousarray(m.reshape(D + 1, 8 * E))

    qf = q.reshape(-1).astype(np.float16)
    kf = k.reshape(-1).astype(np.float16)
    vf = v.reshape(-1).astype(np.float16)
    C = S * D
    in_maps = []
    for i in range(NCORES):
        lo = 2 * i * C
        hi = (2 * i + 2) * C
        in_maps.append(
            dict(
                q_in=np.ascontiguousarray(qf[lo:hi].reshape(2 * S, D)),
                k_in=np.ascontiguousarray(kf[lo:hi].reshape(2 * S, D)),
                v_in=np.ascontiguousarray(vf[lo:hi].reshape(2 * S, D)),
                g_in=g,
                m_in=m2,
            )
        )
    return in_maps


def kernel(q, k, v, Wq, Wk, Wv, Wfc, bfc, _trace=False):
    nc = _get_nc()
    in_maps = make_in_maps(q, k, v, Wq, Wk, Wv, Wfc, bfc)
    tmpdir = None
    if _trace:
        import os, shutil
        tmpdir = "/tmp/bassrun"
        shutil.rmtree(tmpdir, ignore_errors=True)
        os.makedirs(tmpdir, exist_ok=True)
    res = bass_utils.run_bass_kernel_spmd(
        nc, in_maps, core_ids=list(range(NCORES)), trace=_trace, tmpdir=tmpdir
    )
    out = np.concatenate([res.results[i]["out"] for i in range(NCORES)], axis=0)
    kernel.last_exec_time_ns = res.exec_time_ns
    kernel.last_results = res
    return out.reshape(S, 2, E)
